# revision 17
# baseline (speedup 1.0000x reference)
"""Trainium2 Bass kernel for nn_CapsuleMappingTiny.

Reference computation (per batch item b):
    kn = l2norm(k[b])             # rows, D=256
    qn = l2norm(q[b])
    M  = kn @ qn.T                # [N, N]
    M  = LN1(M) over last axis (gamma1, beta1, eps=1e-3)
    out[b] = LN2(M @ v[b]) over last axis (gamma2, beta2, eps=1e-3)

Key algebraic restructuring (valid when gamma1==1, beta1==0, which the
problem's input spec guarantees via fill=ones/zeros):

  * LayerNorm over the last axis is invariant to any positive per-row
    scale of its input (up to eps, which is negligible here because the
    row variances are O(1000) vs eps=1e-3).  The l2-normalization of k
    is a per-row scale of M, and LN1's rsqrt(var+eps) factor is a
    per-row scale of M@v -- both cancel inside LN2.
  * The mean subtraction of LN1 survives:
        LN2( (M - rowmean(M) 1^T) @ v )
      = LN2( k @ (qn^T v)  -  (k @ (qn^T 1) / N) outer (1^T v) )
    so the [N,N] matrix M never needs to be materialized: everything
    factors through D x D matrices (8x fewer FLOPs).

Sharded data-parallel over batch B=32 across 8 NeuronCores (4 items per
core), no communication.  Matmul operands are bf16 (full-rate PE, fp32
PSUM accumulation); inputs are cast to bf16 on the host so DMA moves
half the bytes.
"""

import numpy as np
import ml_dtypes

import concourse.bass as bass
import concourse.bacc as bacc_mod
import concourse.mybir as mybir
import concourse.tile as tile
from concourse.bass_utils import run_bass_kernel_spmd
from concourse.masks import make_identity

F32 = mybir.dt.float32
BF16 = mybir.dt.bfloat16
AF = mybir.ActivationFunctionType
ALU = mybir.AluOpType

B, N, D = 32, 1024, 256
NCORES = 8
BPC = B // NCORES        # batch items per core
P = 128                  # partitions
NT = N // P              # 8 row-tiles per batch item
DT = D // P              # 2 d-chunks
L2_EPS = 1e-12
LN_EPS = 1e-3


def build_fast():
    """Bass program for the gamma=1/beta=0 case (the spec's fill values)."""
    nc = bacc_mod.Bacc()
    kd = nc.declare_dram_parameter("k", [BPC, N, D], BF16, isOutput=False)
    qd = nc.declare_dram_parameter("q", [BPC, N, D], BF16, isOutput=False)
    vd = nc.declare_dram_parameter("v", [BPC, N, D], BF16, isOutput=False)
    outd = nc.declare_dram_parameter("out", [BPC, N, D], BF16, isOutput=True)

    with tile.TileContext(nc) as tc:
        with (
            tc.tile_pool(name="const", bufs=1) as const,
            tc.tile_pool(name="inp", bufs=1) as inp,
            tc.tile_pool(name="data", bufs=3) as data,
            tc.tile_pool(name="ps_tp", bufs=1, space="PSUM") as ps_tp,
            tc.tile_pool(name="ps_g", bufs=2, space="PSUM") as ps_g,
            tc.tile_pool(name="ps_s", bufs=2, space="PSUM") as ps_s,
            tc.tile_pool(name="ps_h", bufs=3, space="PSUM") as ps_h,
        ):
            ident = const.tile([P, P], BF16)
            make_identity(nc, ident)
            ones_row = const.tile([1, P], BF16)
            nc.gpsimd.memset(ones_row, 1.0)
            eps_l2 = const.tile([P, 1], F32)
            nc.gpsimd.memset(eps_l2, L2_EPS)
            eps_ln = const.tile([P, 1], F32)
            nc.gpsimd.memset(eps_ln, LN_EPS)

            # Warm-up transpose: absorbs the gpsimd (identity memset) wait on
            # the PE clock so later transposes never need it.  Transpose-mode
            # matmuls lower to a separate LDWEIGHTS which has a single
            # sync-wait slot, so each real transpose may carry at most ONE
            # unmet dependency.
            tp0 = ps_tp.tile([P, 512], BF16, tag="tp", name="tp0")
            nc.tensor.transpose(tp0[:, 0:P], ident, ident)

            # ---- prefetch ALL inputs up front (pure loads, no waits) -------
            # Keeps the SP DMA queue from blocking later batches' loads
            # behind earlier batches' stores.
            q_a, k_a, v_a = {}, {}, {}
            for b in range(BPC):
                q_a[b] = inp.tile([P, NT, D], BF16, name=f"qa{b}")
                k_a[b] = inp.tile([P, NT, D], BF16, name=f"ka{b}")
                v_a[b] = inp.tile([P, NT, D + 1], BF16, name=f"va{b}")
                nc.sync.dma_start(
                    out=q_a[b],
                    in_=qd[b].rearrange("(j p) d -> p j d", p=P))
                nc.sync.dma_start(
                    out=k_a[b],
                    in_=kd[b].rearrange("(j p) d -> p j d", p=P))
                nc.sync.dma_start(
                    out=v_a[b][:, :, 0:D],
                    in_=vd[b].rearrange("(j p) d -> p j d", p=P))
                nc.gpsimd.memset(v_a[b][:, :, D:D + 1], 1.0)

            for b in range(BPC):
                # ---- l2-normalize q -> qn (bf16) ---------------------------
                ss = data.tile([P, NT], F32, tag="ss", name="ss")
                sq_scratch = data.tile([P, D], F32, tag="sq_scratch",
                                       name="sq_scratch")
                sq_scr2 = data.tile([P, D], F32, tag="sq_scr2",
                                    name="sq_scr2")
                for j in range(NT):
                    if j % 2 == 0:
                        nc.scalar.activation(
                            out=sq_scratch, in_=q_a[b][:, j, :], func=AF.Square,
                            accum_out=ss[:, j:j + 1],
                        )
                    else:
                        nc.vector.scalar_tensor_tensor(
                            out=sq_scr2, in0=q_a[b][:, j, :], scalar=1.0,
                            in1=q_a[b][:, j, :], op0=ALU.mult, op1=ALU.mult,
                            accum_out=ss[:, j:j + 1],
                        )
                rln = data.tile([P, NT], F32, tag="rln", name="rln")
                nc.scalar.activation(out=rln, in_=ss, func=AF.Sqrt, bias=eps_l2)
                rinv = data.tile([P, NT], F32, tag="rinv", name="rinv")
                nc.vector.reciprocal(out=rinv, in_=rln)
                qn_t = [data.tile([P, D], BF16, tag=f"qn{j}", name=f"qn{j}")
                        for j in range(NT)]
                for j in range(NT):
                    if j % 2 == 0:
                        nc.gpsimd.tensor_scalar_mul(
                            out=qn_t[j], in0=q_a[b][:, j, :], scalar1=rinv[:, j:j + 1]
                        )
                    else:
                        nc.scalar.activation(
                            out=qn_t[j], in_=q_a[b][:, j, :], func=AF.Copy,
                            scale=rinv[:, j:j + 1],
                        )

                # ---- transpose k -> kT[c] = [P(d), N] ----------------------
                kT = [data.tile([P, N], BF16, tag=f"kT{c}", name=f"kT{c}")
                      for c in range(DT)]
                for c in range(DT):
                    for g in range(2):          # two groups of 4 blocks
                        tp = ps_tp.tile([P, 512], BF16, tag="tp", name="tp")
                        # Dummy transpose: first writer of the slot, absorbs
                        # the PSUM slot-release wait (same-engine WAW with the
                        # real transposes needs no semaphore).
                        nc.tensor.transpose(tp[:, 0:P], ident, ident)
                        for j4 in range(4):
                            j = g * 4 + j4
                            nc.tensor.transpose(
                                tp[:, j4 * P:(j4 + 1) * P],
                                k_a[b][:, j, c * P:(c + 1) * P],
                                ident,
                            )
                        nc.scalar.copy(
                            out=kT[c][:, g * 512:(g + 1) * 512], in_=tp
                        )

                # ---- G = qn^T @ [v | 1]  ([D, D+1], 2 chunks) --------------
                G_sb = [data.tile([P, D + 1], BF16, tag=f"G{c}", name=f"G{c}")
                        for c in range(DT)]
                for c in range(DT):
                    Gp = ps_g.tile([P, D + 1], F32, tag="Gp", name="Gp")
                    for j in range(NT):
                        nc.tensor.matmul(
                            Gp,
                            lhsT=qn_t[j][:, c * P:(c + 1) * P],
                            rhs=v_a[b][:, j, 0:D + 1],
                            start=(j == 0), stop=(j == NT - 1),
                        )
                    nc.vector.tensor_copy(out=G_sb[c], in_=Gp)

                # ---- s = 1^T v  -> broadcast to [P, D] ---------------------
                sp = ps_s.tile([1, D], F32, tag="smix", name="sp")
                for j in range(NT):
                    nc.tensor.matmul(
                        sp,
                        lhsT=v_a[b][:, j, D:D + 1],
                        rhs=v_a[b][:, j, 0:D],
                        start=(j == 0), stop=(j == NT - 1),
                    )
                s_sb = data.tile([1, D], BF16, tag="s_sb", name="s_sb")
                nc.vector.tensor_copy(out=s_sb, in_=sp)
                sbp = ps_s.tile([P, D], F32, tag="smix", name="sbp")
                nc.tensor.matmul(
                    sbp, lhsT=ones_row, rhs=s_sb, start=True, stop=True
                )
                s_bc = data.tile([P, D], F32, tag="s_bc", name="s_bc")
                nc.vector.tensor_copy(out=s_bc, in_=sbp)

                # ---- H = k @ G per row-chunk + LN2 epilogue ----------------
                negmean = data.tile([P, NT], F32, tag="negmean", name="negmean")
                mv = data.tile([P, NT, 2], F32, tag="mv", name="mv")
                st6 = data.tile([P, NT, 6], F32, tag="st6", name="st6")
                sd2 = data.tile([P, NT], F32, tag="sd2", name="sd2")
                r2 = data.tile([P, NT], F32, tag="r2", name="r2")
                o_a = data.tile([P, NT, D], BF16, tag="o_a", name="o_a")
                pre_t = [data.tile([P, 2, D], F32, tag=f"pre{h}", name=f"pre{h}")
                         for h in range(NT // 2)]
                for m in range(NT):
                    Hp = ps_h.tile([P, D + 1], F32, tag="Hp", name="Hp")
                    for c in range(DT):
                        nc.tensor.matmul(
                            Hp,
                            lhsT=kT[c][:, m * P:(m + 1) * P],
                            rhs=G_sb[c],
                            start=(c == 0), stop=(c == DT - 1),
                        )
                    # negmean_m = -(k qbar)/N   (ACT: copy with scale)
                    nc.scalar.activation(
                        out=negmean[:, m:m + 1], in_=Hp[:, D:D + 1],
                        func=AF.Copy, scale=-1.0 / N,
                    )
                    # pre = (s_bc * negmean) + H2
                    pre = pre_t[m // 2][:, m % 2, :]
                    nc.vector.scalar_tensor_tensor(
                        out=pre, in0=s_bc, scalar=negmean[:, m:m + 1],
                        in1=Hp[:, 0:D], op0=ALU.mult, op1=ALU.add,
                    )
                    nc.vector.bn_stats(out=st6[:, m, :], in_=pre)
                    nc.vector.bn_aggr(out=mv[:, m, :], in_=st6[:, m, :])
                    if m % 2 == 1:
                        nc.scalar.activation(
                            out=sd2[:, m - 1:m + 1], in_=mv[:, m - 1:m + 1, 1],
                            func=AF.Sqrt, bias=eps_ln,
                        )
                        nc.vector.reciprocal(
                            out=r2[:, m - 1:m + 1], in_=sd2[:, m - 1:m + 1]
                        )
                        for mm in (m - 1, m):
                            eng = nc.gpsimd
                            eng.tensor_scalar(
                                out=o_a[:, mm, :],
                                in0=pre_t[mm // 2][:, mm % 2, :],
                                scalar1=mv[:, mm, 0:1],
                                scalar2=r2[:, mm:mm + 1],
                                op0=ALU.subtract, op1=ALU.mult,
                            )
                nc.sync.dma_start(
                    out=outd[b].rearrange("(j p) d -> p j d", p=P), in_=o_a
                )
    nc.finalize()
    return nc


_CACHE = {}


def _get_nc():
    if "fast" not in _CACHE:
        _CACHE["fast"] = build_fast()
    return _CACHE["fast"]


def _kernel_hw_fast(k, q, v):
    nc = _get_nc()
    core_ids = list(range(NCORES))
    bf = ml_dtypes.bfloat16
    in_maps = []
    for c in core_ids:
        sl = slice(c * BPC, (c + 1) * BPC)
        in_maps.append({
            "k": np.ascontiguousarray(k[sl]).astype(bf),
            "q": np.ascontiguousarray(q[sl]).astype(bf),
            "v": np.ascontiguousarray(v[sl]).astype(bf),
        })
    res = run_bass_kernel_spmd(nc, in_maps, core_ids)
    return np.concatenate(
        [res.results[c]["out"].astype(np.float32) for c in core_ids], axis=0
    )


def _kernel_numpy_general(k, q, v, gamma1, beta1, gamma2, beta2):
    """Exact reference semantics; fallback for non-trivial gamma/beta."""
    def l2n(x):
        sq = np.sum(x * x, axis=-1, keepdims=True)
        return x / np.sqrt(np.maximum(sq, L2_EPS))

    def ln(x, g, b):
        mu = x.mean(axis=-1, keepdims=True)
        var = ((x - mu) ** 2).mean(axis=-1, keepdims=True)
        return (x - mu) / np.sqrt(var + LN_EPS) * g + b

    kn = l2n(k.astype(np.float64))
    qn = l2n(q.astype(np.float64))
    m = np.einsum("bkd,bqd->bkq", kn, qn)
    m = ln(m, gamma1.astype(np.float64), beta1.astype(np.float64))
    out = np.einsum("bkq,bqd->bkd", m, v.astype(np.float64))
    out = ln(out, gamma2.astype(np.float64), beta2.astype(np.float64))
    return out.astype(np.float32)


def kernel(k, q, v, gamma1, beta1, gamma2, beta2):
    k = np.asarray(k, dtype=np.float32)
    q = np.asarray(q, dtype=np.float32)
    v = np.asarray(v, dtype=np.float32)
    trivial = (
        np.all(np.asarray(gamma1) == 1.0) and np.all(np.asarray(beta1) == 0.0)
        and np.all(np.asarray(gamma2) == 1.0) and np.all(np.asarray(beta2) == 0.0)
    )
    if trivial:
        return _kernel_hw_fast(k, q, v)
    return _kernel_numpy_general(k, q, v, gamma1, beta1, gamma2, beta2)
